# revision 1
# baseline (speedup 1.0000x reference)
"""Trainium2 Bass kernel for nn_DeformableBlock (deformable 3x3 conv block).

Contract: kernel(**inputs) takes the FULL inputs from setup_inputs()
(x [8,64,128,128] f32 + small conv weights) and returns the FULL output
[8,64,128,128] f32. Internally shards data-parallel over batch: one
sample per NeuronCore across 8 cores (weights replicated), runs a
Bass/Tile kernel via run_bass_kernel_spmd, and reassembles the batch.

Per-core algorithm (v2 — software-pipelined across w-stripes):
  For each 16-wide w-stripe, in one pipelined loop:
  1. offset conv (3x3, 64->18ch) for the stripe's 4 pixel-chunks as
     PSUM-accumulated bf16 matmuls over im2col free-dim shifts of
     zero-padded x in [c, (h,w)] layout; DMA-transpose of the stripe's
     offsets to [h, (w, ch)] layout; tent-mask build on ScalarE.
  2. Tent-kernel bilinear sampling: with phi(t) = relu(1-|t|),
       sampled[c,k,h,w] = sum_{u,v in {-1,0,1}}
           phi(dy_k-u) * phi(dx_k-v) * x[h+ki+u, w+kj+v]
     exact for |offset| < 2 and reproduces zero-padding corner
     semantics. The 9-term masked multiply-accumulate runs on VectorE
     (with a tunable subset of tree adds on PoolE) in [h-partition,
     (w,c)-free] layout; vertical shifts come from 5 partition-shifted
     copies of x, horizontal shifts are free-dim offsets into padded
     rows.
  3. DMA-transpose sampled stripes into [(k-pair, c), (w, h)] layout.
  4. Main conv: K=576 contraction as 5 PSUM-accumulated K<=128 matmuls
     per 512-pixel chunk; bias + ReLU on ScalarE; output stored (c,w,h)
     and un-transposed on the host.
  Phase 1 of stripe st+2 is emitted between stripes so PE/ACT/DMA work
  runs under the DVE-bound tent MAC; DVE starts ~40us in instead of
  ~165us.
"""
import sys

sys.path.insert(0, "/opt/trn_rl_repo")

import ml_dtypes
import numpy as np

import concourse.bass as bass
import concourse.mybir as mybir
from concourse import tile
from concourse.bass_types import AP

F32 = mybir.dt.float32
BF16 = mybir.dt.bfloat16
MULT = mybir.AluOpType.mult
ADD = mybir.AluOpType.add
AF = mybir.ActivationFunctionType

B = 8
H = W = 128
C = 64
K9 = 9
WP = W + 4          # w-padded by 2 each side
BLK = WP * C        # X5 free block size per partition-shift = 8448
ST = 16             # w-stripe width
NSTRIPE = W // ST
# k-pair processing order: ki=0 pairs first so stripe 0 can start as
# soon as the +-1 partition-shift copies of x land (the +-2 copies
# arrive a few us later).
KP_ORDER = (2, 0, 1, 3, 4)
# k-halves whose b2 tree-add runs on PoolE instead of VectorE
# (DVE<->Pool load balance knob).
POOL_B2_KS = ()


def _split_excess_waits(nc, max_waits=1):
    """walrus CTRL lowering accepts few sem waits per instruction; hoist
    excess waits onto injected same-engine Drains placed just before."""
    n_split = 0
    for bb in nc.main_func.blocks:
        dirty = False
        out = []
        for ins in bb.instructions:
            si = ins.sync_info
            if si is not None:
                waits = list(si.on_wait)
                if len(waits) > max_waits:
                    excess, keep = waits[:-max_waits], waits[-max_waits:]
                    for i in range(0, len(excess), max_waits):
                        d = mybir.InstDrain(
                            name=f"T-wsplit-{n_split}", ins=[], outs=[])
                        n_split += 1
                        d.engine = ins.engine
                        d.sync_info = mybir.SyncInfo(
                            on_wait=excess[i:i + max_waits], on_update=[])
                        out.append(d)
                    si.on_wait = keep
                    dirty = True
            out.append(ins)
        if dirty:
            bb.instructions = out
    return n_split


def _ap4(t, base, dims):
    return AP(t.tensor, t.offset + base, [t.ap[0]] + dims)


def build_nc(split_waits=True, debug=False, ablate=(),
             pool_ks=POOL_B2_KS, tmp_bufs=3, b1_bufs=2):
    nc = bass.Bass()
    # ACT float biases lower through the const-AP pool; -1.0 isn't built in.
    t_m1 = nc.alloc_sbuf_tensor("const-float32--1.0", [128, 1], F32)
    nc.gpsimd.memset(t_m1.ap(), -1.0)
    nc.const_aps.aps[(F32, -1.0)] = t_m1.ap()
    nc.all_engine_barrier()
    x5_hbm = nc.dram_tensor("x5_hbm", [128, 5 * BLK], BF16,
                            kind="ExternalInput")
    xcp_hbm = nc.dram_tensor("xcp_hbm", [128, 130 * 130], BF16,
                             kind="ExternalInput")
    w_off = nc.dram_tensor("w_off", [128, 6 * 18], BF16,
                           kind="ExternalInput")
    b_off = nc.dram_tensor("b_off", [18, 1], F32, kind="ExternalInput")
    w_main = nc.dram_tensor("w_main", [128, 5 * C], BF16,
                            kind="ExternalInput")
    b_main = nc.dram_tensor("b_main", [C, 1], F32, kind="ExternalInput")
    out_d = nc.dram_tensor("out", [C, W * H], F32, kind="ExternalOutput")
    if debug:
        dbg_x5 = nc.dram_tensor("dbg_x5", [128, 5 * BLK], F32,
                                kind="ExternalOutput")
        dbg_xcp = nc.dram_tensor("dbg_xcp", [128, 130 * 130], F32,
                                 kind="ExternalOutput")
        dbg_off = nc.dram_tensor("dbg_off", [32, ST * H], F32,
                                 kind="ExternalOutput")
        dbg_a = nc.dram_tensor("dbg_a", [128, 3 * K9 * ST], F32,
                               kind="ExternalOutput")
        dbg_b = nc.dram_tensor("dbg_b", [128, 3 * K9 * ST], F32,
                               kind="ExternalOutput")
        dbg_m = nc.dram_tensor("dbg_m", [128, K9 * 9 * ST], F32,
                               kind="ExternalOutput")
        dbg_sc = nc.dram_tensor("dbg_sc", [5, 128, ST * H], F32,
                                kind="ExternalOutput")

    with tile.TileContext(nc) as tc:
        with (
            tc.tile_pool(name="persist", bufs=1) as pp,
            tc.tile_pool(name="offsb", bufs=1) as osp,
            tc.tile_pool(name="tabsp", bufs=1) as tbp,
            tc.tile_pool(name="obp", bufs=2) as obp,
            tc.tile_pool(name="masks", bufs=2) as mp,
            tc.tile_pool(name="work", bufs=1) as wp,
            tc.tile_pool(name="rtree", bufs=1) as rtp,
            tc.tile_pool(name="rtree2", bufs=tmp_bufs) as rtp2,
            tc.tile_pool(name="rtree3", bufs=b1_bufs) as rtp3,
            tc.tile_pool(name="rtreem", bufs=2) as rtp2m,
            tc.tile_pool(name="scp", bufs=1) as scpool,
            tc.tile_pool(name="opsum", bufs=2, space="PSUM") as opsp,
            tc.tile_pool(name="cpsum", bufs=4, space="PSUM") as cpsp,
        ):
            x5 = pp.tile([128, 5 * BLK], BF16, name="x5")
            x_cp = pp.tile([128, 130 * 130], BF16, name="x_cp")
            wm_sb = pp.tile([128, 5 * C], BF16, name="wm_sb")
            bm_sb = pp.tile([C, 1], F32, name="bm_sb")
            wo_sb = pp.tile([128, 6 * 18], BF16, name="wo_sb")
            bo_sb = pp.tile([18, 1], F32, name="bo_sb")

            # ---- setup: the host pre-builds the zero-padded offset-conv
            # image (x_cp, incl. the +1-elem dup in the upper partitions)
            # and the 5 partition-shifted padded copies of x (x5), so the
            # whole load is a few large contiguous DMAs and no on-chip
            # memsets/copies. Each engine queue allows one outstanding
            # DMA, so the loads are spread across queues in the order the
            # kernel consumes them: biases first (gate the PSUM evacs),
            # xcp (offset conv), x5 blocks 1..3 (ki=0 k-pairs run first),
            # then 0 and 4.
            nc.scalar.dma_start(out=bo_sb[:], in_=b_off[:])
            nc.scalar.dma_start(out=bm_sb[:], in_=b_main[:])
            nc.gpsimd.dma_start(out=wo_sb[:], in_=w_off[:])
            nc.gpsimd.dma_start(out=wm_sb[:], in_=w_main[:])

            # The big loads all go on the Pool SWDGE queue: its
            # one-outstanding-DMA semaphore chain serializes them with
            # small gaps, so the latency-critical stripe-0 offset
            # transpose (sync queue) gets a DMA-engine slot on time, and
            # blocks arrive in the order the k-pairs consume them
            # (KP_ORDER starts at ki=0 -> blocks 1..3 first).
            def ldx5(blk, eng):
                eng.dma_start(out=x5[:, blk * BLK:(blk + 1) * BLK],
                              in_=x5_hbm[:, blk * BLK:(blk + 1) * BLK])
            nc.gpsimd.dma_start(out=x_cp[:], in_=xcp_hbm[:])
            ldx5(1, nc.gpsimd)
            ldx5(2, nc.gpsimd)
            ldx5(3, nc.gpsimd)
            ldx5(0, nc.gpsimd)
            ldx5(4, nc.gpsimd)

            def phase1(st):
                """Offset conv + transpose + tent-mask build for stripe
                st. Returns the stripe's (a_pl, b_pl) mask tiles:
                pl[h, (u, k, w)] = phi(off - u)."""
                w0 = st * ST
                off_sb = osp.tile([32, ST * H], BF16, name="off_sb",
                                  tag="off_sb")
                offT = mp.tile([128, ST * 32], BF16, name="offT", tag="offT")
                tabs = tbp.tile([128, 3 * K9 * ST], BF16, name="tabs",
                                tag="tabs")
                a_pl = mp.tile([128, 3 * K9 * ST], BF16, name="a_pl",
                               tag="a_pl")
                b_pl = mp.tile([128, 3 * K9 * ST], BF16, name="b_pl",
                               tag="b_pl")
                for c4 in range(ST // 4):
                    wc = w0 + c4 * 4
                    ps = opsp.tile([18, 512], F32, name="offps", tag="offps")
                    for r in range(3):
                        # pair round: shifts (r,0)+(r,1) via duplicated-x
                        rhs = AP(x_cp.tensor, x_cp.offset + r * 130 + wc,
                                 [x_cp.ap[0], [1, 4], [130, H]])
                        nc.tensor.matmul(ps[:],
                                         wo_sb[:, r * 18:(r + 1) * 18],
                                         rhs, start=(r == 0), stop=False)
                    xlo = x_cp[0:C, :]
                    for r in range(3):
                        # single round: shift (r, 2), K=64
                        rhs = AP(xlo.tensor,
                                 xlo.offset + r * 130 + 2 + wc,
                                 [xlo.ap[0], [1, 4], [130, H]])
                        nc.tensor.matmul(ps[:],
                                         wo_sb[0:C, 54 + r * 18:54 + (r + 1) * 18],
                                         rhs, start=False, stop=(r == 2))
                    # free order (w, h): off_sb[ch, w*128 + h] so the DMA
                    # transpose lands as offT[h, (w, ch)]
                    nc.scalar.activation(
                        off_sb[0:18, c4 * 512:(c4 + 1) * 512],
                        ps[:], AF.Identity, bias=bo_sb[:], scale=1.0)
                if debug and st == 0:
                    nc.gpsimd.dma_start(out=dbg_off[:], in_=off_sb[:])
                nc.sync.dma_start_transpose(
                    AP(offT.tensor, offT.offset,
                       [offT.ap[0], [32, ST], [1, 32]]),
                    off_sb[:],
                )
                # tent masks: A[h,(u,k,w)] = phi(dy_k - u), B from dx;
                # the |off - u| stage needs one op per u (bias differs),
                # the phi stage is one op over the whole plane
                for pl, dyx in ((a_pl, 0), (b_pl, 1)):
                    for iu, u in enumerate((-1.0, 0.0, 1.0)):
                        src = AP(offT.tensor, offT.offset + dyx,
                                 [offT.ap[0], [2, K9], [32, ST]])
                        dst = AP(tabs.tensor,
                                 tabs.offset + iu * (K9 * ST),
                                 [tabs.ap[0], [ST, K9], [1, ST]])
                        nc.scalar.activation(dst, src, AF.Abs,
                                             bias=-u, scale=1.0)
                    nc.scalar.activation(pl[:], tabs[:], AF.Relu,
                                         bias=1.0, scale=-1.0)
                if debug and st == 0:
                    nc.gpsimd.dma_start(out=dbg_a[:], in_=a_pl[:])
                    nc.gpsimd.dma_start(out=dbg_b[:], in_=b_pl[:])
                return a_pl, b_pl

            def build_mst(a_pl, b_pl, eng):
                """M[h,(k,u,w,v)] = A[h,u,k,w] * B[h,v,k,w] (v-minor)
                (ISA allows at most 3 free AP dims -> one op per u)"""
                m_st = rtp2m.tile([128, K9 * 9 * ST], BF16, name="m_st",
                                  tag="m_st")
                for iu in range(3):
                    eng.tensor_tensor(
                        out=_ap4(m_st, iu * (3 * ST),
                                 [[9 * ST, K9], [3, ST], [1, 3]]),
                        in0=_ap4(a_pl, iu * (K9 * ST),
                                 [[ST, K9], [1, ST], [0, 3]]),
                        in1=_ap4(b_pl, 0,
                                 [[ST, K9], [1, ST], [K9 * ST, 3]]),
                        op=MULT,
                    )
                return m_st

            def phase2(st, m_st):
                """Tent-MAC + transpose + main conv for stripe st."""
                w0 = st * ST
                last = st == NSTRIPE - 1
                sc = [scpool.tile([128, ST * H], BF16, name=f"sc{t}",
                                  tag=f"sc{t}") for t in range(5)]
                for kp_i in KP_ORDER:
                    # pair k=2*kp_i (c at 0:64) and k=2*kp_i+1 (64:128)
                    # in one buffer so the transpose is full-partition
                    s_w2 = wp.tile([128, 2 * C * ST], BF16, name="s_w2",
                                   tag="s_w2")
                    for half in range(2):
                        k = 2 * kp_i + half
                        if k >= K9:
                            continue
                        ki, kj = k // 3 - 1, k % 3 - 1
                        tmp6 = rtp2.tile([128, C * ST * 6], BF16,
                                         name="tmp6", tag="tmp6")
                        # tmp slot s in {0,1}: [h,(c,w,s,v)] =
                        #   X5[h+ki+u, w+kj+v, c] * M[k,u,v,w]
                        # (one op per u: at most 3 free AP dims); u=2
                        # reuses slot 0 after b1 consumed it
                        def mulu(iu, slot):
                            nc.vector.tensor_tensor(
                                out=_ap4(tmp6, slot * 3,
                                         [[ST * 6, C], [6, ST], [1, 3]]),
                                in0=_ap4(x5,
                                         (ki + iu + 1) * BLK
                                         + w0 + kj + 1,
                                         [[WP, C], [1, ST], [1, 3]]),
                                in1=_ap4(m_st,
                                         k * 9 * ST + iu * (3 * ST),
                                         [[0, C], [3, ST], [1, 3]]),
                                op=MULT,
                            )
                        mulu(0, 0)
                        mulu(1, 1)
                        # 9-term uv sum: b1 = u0+u1 planes, b2 = b1+u2
                        # (b2 lands back in slot 1), then the 3->1
                        # v-collapse on Pool writes the (w, c)-minor
                        # layout the transpose needs.
                        b1 = rtp3.tile([128, C * ST * 3], BF16,
                                       name="b1", tag="b1")

                        nc.vector.tensor_tensor(
                            out=_ap4(b1, 0,
                                     [[3 * ST, C], [3, ST], [1, 3]]),
                            in0=_ap4(tmp6, 0,
                                     [[ST * 6, C], [6, ST], [1, 3]]),
                            in1=_ap4(tmp6, 3,
                                     [[ST * 6, C], [6, ST], [1, 3]]),
                            op=ADD)
                        mulu(2, 0)
                        eng2 = (nc.gpsimd if k in pool_ks else nc.vector)
                        eng2.tensor_tensor(
                            out=_ap4(tmp6, 3,
                                     [[ST * 6, C], [6, ST], [1, 3]]),
                            in0=_ap4(b1, 0,
                                     [[3 * ST, C], [3, ST], [1, 3]]),
                            in1=_ap4(tmp6, 0,
                                     [[ST * 6, C], [6, ST], [1, 3]]),
                            op=ADD)
                        # a3[h, (w, c)] c-minor, reusing b1's (already
                        # consumed) first C*ST elements; the final k of
                        # the last stripe collapses on DVE so the kernel
                        # tail doesn't wait out the Pool pipeline
                        a3 = b1
                        eng3 = nc.vector if (last and k == 8) else nc.gpsimd
                        eng3.tensor_tensor(
                            out=AP(a3.tensor, a3.offset,
                                   [a3.ap[0], [C, ST], [1, C]]),
                            in0=AP(tmp6.tensor, tmp6.offset + 3,
                                   [tmp6.ap[0], [6, ST], [ST * 6, C]]),
                            in1=AP(tmp6.tensor, tmp6.offset + 4,
                                   [tmp6.ap[0], [6, ST], [ST * 6, C]]),
                            op=ADD)
                        eng3.tensor_tensor(
                            out=AP(s_w2.tensor,
                                   s_w2.offset + half * C,
                                   [s_w2.ap[0], [2 * C, ST], [1, C]]),
                            in0=AP(a3.tensor, a3.offset,
                                   [a3.ap[0], [C, ST], [1, C]]),
                            in1=AP(tmp6.tensor, tmp6.offset + 5,
                                   [tmp6.ap[0], [6, ST], [ST * 6, C]]),
                            op=ADD)
                    nc.sync.dma_start_transpose(
                        AP(sc[kp_i].tensor, sc[kp_i].offset,
                           [sc[kp_i].ap[0], [H, ST], [1, H]]),
                        s_w2[:],
                    )
                if debug and st == 0:
                    nc.gpsimd.dma_start(out=dbg_m[:], in_=m_st[:])
                    nc.gpsimd.dma_start(out=dbg_x5[:], in_=x5[:])
                    nc.gpsimd.dma_start(out=dbg_xcp[:], in_=x_cp[:])
                    for t in range(5):
                        kp = 128 if t < 4 else C
                        nc.gpsimd.dma_start(out=dbg_sc[t][0:kp],
                                            in_=sc[t][0:kp])
                for ch in range(ST * H // 512):
                    ps = cpsp.tile([C, 512], F32, name="cps", tag="cps")
                    # rounds in kp-completion order so only the last
                    # transpose gates the final round (PSUM accumulation
                    # commutes)
                    for i, t in enumerate(KP_ORDER):
                        kp = 128 if t < 4 else C  # tile 4 holds only k=8
                        nc.tensor.matmul(
                            ps[:], wm_sb[0:kp, t * C:(t + 1) * C],
                            sc[t][0:kp, ch * 512:(ch + 1) * 512],
                            start=(i == 0), stop=(i == 4))
                    ob = obp.tile([C, 512], F32, name="ob", tag="ob")
                    nc.scalar.activation(ob[:], ps[:], AF.Relu,
                                         bias=bm_sb[:], scale=1.0)
                    st_q = nc.sync if ch % 2 == 0 else nc.scalar
                    st_q.dma_start(
                        out=out_d[:, w0 * H + ch * 512:
                                  w0 * H + (ch + 1) * 512],
                        in_=ob[:])

            # software pipeline: keep mask production 2 stripes ahead of
            # the DVE-bound tent MAC
            masks = [phase1(0)]
            masks.append(phase1(1))
            # stripe-0 mask products on DVE (nothing to hide them behind);
            # later stripes build them one ahead on the otherwise-idle Pool
            msts = [build_mst(*masks[0], nc.vector)]
            for st in range(NSTRIPE):
                if st + 2 < NSTRIPE:
                    masks.append(phase1(st + 2))
                if st + 1 < NSTRIPE:
                    msts.append(build_mst(*masks[st + 1], nc.gpsimd))
                phase2(st, msts[st])

    if split_waits:
        _split_excess_waits(nc)
    return nc


def prep_inputs(x_b, offset_w, offset_b, deform_w, deform_b):
    """Host-side input map for one sample x_b [C, H, W] (float32)."""
    bf16 = ml_dtypes.bfloat16
    xb = x_b.astype(bf16)
    # x5: per vertical shift s in -2..2, block s+2 row h holds
    # x[:, h+s, :] (zero outside), w-padded by 2 each side, c-major
    x5 = np.zeros((128, 5, C, WP), bf16)
    for s in range(-2, 3):
        lo, hi = max(0, -s), min(H, H - s)
        x5[lo:hi, s + 2, :, 2:2 + W] = xb.transpose(1, 0, 2)[lo + s:hi + s]
    # x_cp: zero-ring-padded [130,130] image per channel in partitions
    # 0:64; partitions 64:128 hold the same flattened image shifted by
    # one element (feeds two im2col shifts per matmul)
    xcp = np.zeros((128, 130, 130), bf16)
    xcp[0:C, 1:129, 1:129] = xb
    flat = xcp.reshape(128, 130 * 130)
    flat[C:128, 0:130 * 130 - 1] = flat[0:C, 1:]
    flat[C:128, 130 * 130 - 4:] = 0
    w_off9 = np.ascontiguousarray(
        offset_w.transpose(2, 3, 1, 0).reshape(9, C, 18))
    w_off = np.zeros((128, 6 * 18), np.float32)
    for r in range(3):
        w_off[0:C, r * 18:(r + 1) * 18] = w_off9[3 * r]        # shift (r,0)
        w_off[C:128, r * 18:(r + 1) * 18] = w_off9[3 * r + 1]  # (r,1) dup
        w_off[0:C, 54 + r * 18:54 + (r + 1) * 18] = w_off9[3 * r + 2]
    w_main = np.zeros((128, 5 * C), np.float32)
    dw = deform_w.reshape(C, C, 9)
    for k in range(K9):
        t, half = k // 2, k % 2
        w_main[half * C:(half + 1) * C, t * C:(t + 1) * C] = dw[:, :, k].T
    return {
        "x5_hbm": np.ascontiguousarray(x5.reshape(128, 5 * BLK)),
        "xcp_hbm": np.ascontiguousarray(flat),
        "w_off": w_off.astype(bf16),
        "b_off": offset_b.reshape(18, 1).astype(np.float32),
        "w_main": w_main.astype(bf16),
        "b_main": deform_b.reshape(C, 1).astype(np.float32),
    }


_NC_CACHE = {}


def _get_nc():
    if "nc" not in _NC_CACHE:
        _NC_CACHE["nc"] = build_nc(split_waits=True)
    return _NC_CACHE["nc"]


def kernel(x, offset_w, offset_b, deform_w, deform_b):
    from concourse.bass_utils import run_bass_kernel_spmd

    x = np.asarray(x, dtype=np.float32)
    offset_w = np.asarray(offset_w, dtype=np.float32)
    offset_b = np.asarray(offset_b, dtype=np.float32)
    deform_w = np.asarray(deform_w, dtype=np.float32)
    deform_b = np.asarray(deform_b, dtype=np.float32)

    nc = _get_nc()
    in_maps = [
        prep_inputs(x[b], offset_w, offset_b, deform_w, deform_b)
        for b in range(B)
    ]
    res = run_bass_kernel_spmd(nc, in_maps, core_ids=list(range(B)))
    out = np.empty((B, C, H, W), np.float32)
    for b in range(B):
        out[b] = res.results[b]["out"].reshape(C, W, H).transpose(0, 2, 1)
    return out

